# revision 5
# baseline (speedup 1.0000x reference)
"""GNN NodeBlock kernel for 8x TRN2 NeuronCores (v3).

v2 + smaller routing bins: nodes are packed into bins of 32 slots with
a fixed 3x128-token budget (LPT on degree, 27% slack), so the on-device
one-hot build touches 32 columns per token instead of 64 (DVE cost
-40%) and each aggregation matmul streams 32 one-hot columns (PE cost
-40%), for ~20% more edge-stream padding. Engine layout:
  SP/Pool: edge-payload DMA    Pool: ab DMA (SWDGE)
  DVE: one-hot EQ, aggT copy, output bias-add
  Act: relu, output DMA        PE: all matmuls
"""

import heapq

import ml_dtypes
import numpy as np
from contextlib import ExitStack

import concourse.bass as bass
import concourse.tile as tile
from concourse import bacc, mybir
from concourse.bass import AP
from concourse.bass_utils import run_bass_kernel_spmd

N_NODES = 100000
N_EDGES = 1000000
D = 64
NB = 64
LATENT = 32
OUT_DIM = 64

NCORES = 8
NPC = N_NODES // NCORES      # 12500 nodes per core
BIN = 32                     # node slots per bin
K3 = 3                       # 128-token tiles per bin
BTOK = K3 * 128              # 384 edge-token capacity per bin
NBIN = 416                   # bins per core
NSLOT = NBIN * BIN           # 13312 node slots (>= NPC)
NT = NBIN * K3               # 1248 token tiles per core
CAPT = NT * 128              # 159744 token slots per core
SUPB = 32                    # bins per supertile
SUP = SUPB * BIN             # 1024 nodes per supertile
WT = SUPB * K3               # 96 token tiles per supertile
NSUP = NBIN // SUPB          # 13 supertiles

F32 = mybir.dt.float32
F32R = mybir.dt.float32r
BF16 = mybir.dt.bfloat16
F8 = mybir.dt.float8e4
EQ = mybir.AluOpType.is_equal
ADD = mybir.AluOpType.add
Copy = mybir.ActivationFunctionType.Copy
Relu = mybir.ActivationFunctionType.Relu
Identity = mybir.ActivationFunctionType.Identity

_PROG = None

MACRO = 2        # supertiles per DMA macro-batch
# NOTE: GPSIMD/Pool cannot read PSUM on real HW and cannot run is_equal
# (the simulator allows both, neuronxcc rejects them) — Pool is DMA-only.
AGG_DVE = set()        # supertile aggT copies on DVE, rest on Act
OBT_ACT = set()        # supertile obT bias-adds on Act, rest on DVE
OUT_SP = set()         # macro indices whose output store issues from SP
OUT_POOL = set(range(8))  # macro indices whose output store issues from Pool
# (m0, nsup) DMA macro-batches; singles at both ends shorten startup/tail
MACROS = [(0, 1), (1, 2), (3, 2), (5, 2), (7, 2), (9, 2), (11, 1), (12, 1)]


def _bcast(ap, dim, n):
    """Insert a zero-stride dim of size n at free-dim position dim."""
    layout = list(ap.ap)
    layout.insert(1 + dim, [0, n])
    return AP(ap.tensor, ap.offset, layout)


def _build_program(reps=1):
    nc = bacc.Bacc(None, target_bir_lowering=False, debug=True)

    edges_d = nc.dram_tensor("edges_tok", [128, NT, D], F8, kind="ExternalInput")
    ridx_d = nc.dram_tensor("ridx", [128, NT], BF16, kind="ExternalInput")
    ab_d = nc.dram_tensor("ab", [128, NSLOT], BF16, kind="ExternalInput")
    gaT_d = nc.dram_tensor("gaT", [D, NB], F32R, kind="ExternalInput")
    w1n_d = nc.dram_tensor("w1n", [D, LATENT], BF16, kind="ExternalInput")
    w1a_d = nc.dram_tensor("w1a", [D, LATENT], BF16, kind="ExternalInput")
    w1g_d = nc.dram_tensor("w1g", [D, LATENT], F32R, kind="ExternalInput")
    w2_d = nc.dram_tensor("w2", [LATENT, OUT_DIM], F32R, kind="ExternalInput")
    b1_d = nc.dram_tensor("b1c", [LATENT, 1], F32, kind="ExternalInput")
    b2_d = nc.dram_tensor("b2b", [OUT_DIM, 512], F32, kind="ExternalInput")
    b2c_d = nc.dram_tensor("b2c", [OUT_DIM, 1], F32, kind="ExternalInput")
    iotax_d = nc.dram_tensor("iotax", [128, BIN, WT], BF16, kind="ExternalInput")
    out_d = nc.dram_tensor("out", [OUT_DIM, NSLOT], BF16, kind="ExternalOutput")

    with tile.TileContext(nc) as tc:
     # body repeated `reps` times for delta-timing (overhead cancels)
     for _rep in range(reps):
      with ExitStack() as stk:
        persist = stk.enter_context(tc.tile_pool(name="persist", bufs=1))
        gaT = persist.tile([D, NB], F32R)
        w1ng = persist.tile([128, LATENT], BF16)
        w1a = persist.tile([D, LATENT], BF16)
        w1g = persist.tile([D, LATENT], F32R)
        w2 = persist.tile([LATENT, OUT_DIM], F32R)
        b1c = persist.tile([LATENT, 1], F32)
        b2b = persist.tile([OUT_DIM, 512], F32)
        b2c = persist.tile([OUT_DIM, 1], F32)
        iotax = persist.tile([128, BIN, WT], BF16)
        ridx_sb = persist.tile([128, NT], BF16)

        # one-time const loads; ridx/iotax first (they gate the EQ chain).
        # ridx is split so supertile 0's slice lands immediately.
        nc.sync.dma_start(ridx_sb[:, 0:WT], ridx_d[:, 0:WT])
        nc.scalar.dma_start(ridx_sb[:, WT:], ridx_d[:, WT:])
        for eng, sb, dr in ((nc.gpsimd, iotax, iotax_d),
                            (nc.sync, gaT, gaT_d), (nc.sync, w1g, w1g_d),
                            (nc.scalar, w1a, w1a_d), (nc.scalar, w2, w2_d),
                            (nc.scalar, b1c, b1_d),
                            (nc.gpsimd, b2b, b2_d),
                            (nc.gpsimd, b2c, b2c_d)):
            eng.dma_start(sb[:], dr[:])
        nc.gpsimd.dma_start(w1ng[0:D, :], w1n_d[:])

        # G = global_attr @ W1g stacked under W1n as the [128,32] stationary
        with tc.tile_pool(name="psg", bufs=1, space="PSUM") as psg:
            ps_g = psg.tile([NB, LATENT], F32)
            nc.tensor.matmul(ps_g[:], gaT[:], w1g[:], start=True, stop=True)
            nc.scalar.activation(w1ng[D:128, :], ps_g[:], Copy)

        ppool = stk.enter_context(tc.tile_pool(name="pt", bufs=2))
        ohpool = stk.enter_context(tc.tile_pool(name="ohb", bufs=6))
        abpool = stk.enter_context(tc.tile_pool(name="ab", bufs=2))
        aggp = stk.enter_context(tc.tile_pool(name="agg", bufs=2))
        hp = stk.enter_context(tc.tile_pool(name="hp", bufs=2))
        opool = stk.enter_context(tc.tile_pool(name="op", bufs=2))
        psa = stk.enter_context(tc.tile_pool(name="psa", bufs=2, space="PSUM"))
        ps1p = stk.enter_context(tc.tile_pool(name="ps1", bufs=2, space="PSUM"))
        ps2p = stk.enter_context(tc.tile_pool(name="ps2", bufs=2, space="PSUM"))

        # macro-iterations: batches per-DMA fixed cost across supertiles.
        # First macro is a single supertile so the compute chain starts
        # as early as possible.
        macros = MACROS
        for mi, (m0, nsup) in enumerate(macros):
            mwt = nsup * WT
            msup = nsup * SUP
            ab = abpool.tile([128, msup], BF16, name="ab")
            nc.gpsimd.dma_start(ab[:], ab_d[:, SUP * m0:SUP * m0 + msup])

            pt = ppool.tile([128, mwt, D], F8, name="pt")
            nc.sync.dma_start(pt[:], edges_d[:, WT * m0:WT * m0 + mwt, :])

            obT = opool.tile([OUT_DIM, msup], BF16, name="obT")
            # one-hot routing, slot-major [128, BIN, WT]: every operand has
            # a stride-1 2-byte last dim, so DVE runs at 2 elem/cyc. All of
            # the macro's EQs are emitted first so DVE's in-order queue
            # doesn't stall them behind obT adds waiting on ps2.
            ohbTs = []
            for si in range(nsup):
                s = m0 + si
                ohbT = ohpool.tile([128, BIN, WT], BF16, name="ohbT")
                nc.vector.tensor_tensor(
                    ohbT[:], iotax[:],
                    _bcast(ridx_sb[:, WT * s:WT * (s + 1)], 0, BIN), op=EQ)
                ohbTs.append(ohbT)
            for si in range(nsup):
                s = m0 + si
                ohbT = ohbTs[si]

                # aggT[feat, node] accumulated per bin into one PSUM tile
                # (lhsT fp8 payload x strided bf16 one-hot columns)
                ps_agg = psa.tile([D, SUP], F32, name="ps_agg")
                for bi in range(SUPB):
                    for j in range(K3):
                        t = K3 * bi + j
                        nc.tensor.matmul(
                            ps_agg[:, BIN * bi:BIN * (bi + 1)],
                            pt[:, WT * si + t, :], ohbT[:, :, t],
                            start=(j == 0), stop=(j == K3 - 1))
                aggT = aggp.tile([D, SUP], BF16, name="aggT")
                if s in AGG_DVE:
                    nc.vector.tensor_scalar_add(aggT[:], ps_agg[:], 0.0)
                else:
                    nc.scalar.activation(aggT[:], ps_agg[:], Copy)

                for q in range(2):
                    sl = slice(SUP * si + 512 * q, SUP * si + 512 * (q + 1))
                    hsl = slice(512 * q, 512 * (q + 1))
                    ps1 = ps1p.tile([LATENT, 512], F32, name="ps1")
                    nc.tensor.matmul(ps1[:], w1ng[:], ab[:, sl],
                                     start=True, stop=False)
                    nc.tensor.matmul(ps1[:], w1a[:], aggT[:, hsl],
                                     start=False, stop=True)
                    h = hp.tile([LATENT, 512], F32R, name="h")
                    nc.scalar.activation(h[:], ps1[:], Relu, bias=b1c[:])
                    ps2 = ps2p.tile([OUT_DIM, 512], F32, name="ps2")
                    nc.tensor.matmul(ps2[:], w2[:], h[:], start=True, stop=True)
                    if s in OBT_ACT and q == 1:
                        nc.scalar.activation(obT[:, sl], ps2[:], Identity,
                                             bias=b2c[:])
                    else:
                        nc.vector.tensor_tensor(obT[:, sl], ps2[:], b2b[:],
                                                op=ADD)
            out_eng = (nc.sync if mi in OUT_SP
                       else nc.gpsimd if mi in OUT_POOL else nc.scalar)
            out_eng.dma_start(out_d[:, SUP * m0:SUP * m0 + msup], obT[:])

    nc.compile()
    return nc


def _pack_bins(deg):
    """LPT bin-packing: assign each node to a bin, balancing edge load
    with caps of BIN nodes / BTOK edges per bin."""
    bin_of = np.empty(NPC, np.int32)
    slot_of = np.empty(NPC, np.int32)
    counts = np.zeros(NBIN, np.int32)
    loads = np.zeros(NBIN, np.int64)
    heap = [(0, b) for b in range(NBIN)]
    for n in np.argsort(-deg, kind="stable"):
        while True:
            load, b = heapq.heappop(heap)
            if counts[b] < BIN and loads[b] + deg[n] <= BTOK:
                break
        bin_of[n] = b
        slot_of[n] = counts[b]
        counts[b] += 1
        loads[b] += deg[n]
        if counts[b] < BIN:
            heapq.heappush(heap, (int(loads[b]), b))
    return bin_of, slot_of


def _prep_inputs(node_attr, edge_attr, global_attr, W1, b1, W2, b2,
                 receivers_idx, ng_index):
    node_attr = np.asarray(node_attr, np.float32)
    edge_attr = np.asarray(edge_attr, np.float32)
    global_attr = np.asarray(global_attr, np.float32)
    W1 = np.asarray(W1, np.float32)
    b1 = np.asarray(b1, np.float32)
    W2 = np.asarray(W2, np.float32)
    b2 = np.asarray(b2, np.float32)
    receivers_idx = np.asarray(receivers_idx, np.int64)
    ng_index = np.asarray(ng_index, np.int64)

    BF = ml_dtypes.bfloat16
    F8np = mybir.dt.np(F8)
    shared = {
        "gaT": np.ascontiguousarray(global_attr.T),
        "w1n": np.ascontiguousarray(W1[0:D]).astype(BF),
        "w1a": np.ascontiguousarray(W1[D:2 * D]).astype(BF),
        "w1g": np.ascontiguousarray(W1[2 * D:3 * D]),
        "w2": np.ascontiguousarray(W2),
        "b1c": np.ascontiguousarray(b1.reshape(LATENT, 1)),
        "b2b": np.ascontiguousarray(
            np.broadcast_to(b2[:, None], (OUT_DIM, 512))).astype(np.float32),
        "iotax": np.ascontiguousarray(np.broadcast_to(
            np.arange(BIN, dtype=BF)[None, :, None], (128, BIN, WT))),
        "b2c": np.ascontiguousarray(b2.reshape(OUT_DIM, 1)),
    }

    deg_all = np.bincount(receivers_idx, minlength=N_NODES).astype(np.float32)
    escale = (1.0 / np.maximum(deg_all, 1.0))[receivers_idx]

    order = np.argsort(receivers_idx, kind="stable")
    sorted_recv = receivers_idx[order]
    bounds = np.searchsorted(sorted_recv, np.arange(0, N_NODES + 1, NPC))

    in_maps = []
    perms = []
    for k in range(NCORES):
        sel = order[bounds[k]:bounds[k + 1]]
        lrecv = (sorted_recv[bounds[k]:bounds[k + 1]] - k * NPC).astype(np.int64)
        e = sel.size
        deg = np.bincount(lrecv, minlength=NPC)
        bin_of, slot_of = _pack_bins(deg)

        ew = bin_of[lrecv].astype(np.int64)
        ord2 = np.argsort(ew, kind="stable")
        sel2 = sel[ord2]
        lrecv2 = lrecv[ord2]
        ew2 = ew[ord2]
        starts = np.searchsorted(ew2, np.arange(NBIN))
        pos = np.arange(e) - starts[ew2]
        assert e == 0 or pos.max() < BTOK
        tokslot = ew2 * BTOK + pos

        tok = np.zeros((CAPT, D), F8np)
        tok[tokslot] = (edge_attr[sel2] * escale[sel2][:, None]).astype(F8np)
        edges_tok = np.ascontiguousarray(
            tok.reshape(NT, 128, D).transpose(1, 0, 2))
        rx = np.full(CAPT, -1.0, BF)
        rx[tokslot] = slot_of[lrecv2].astype(BF)
        ridx = np.ascontiguousarray(rx.reshape(NT, 128).T)

        perm = np.full(NSLOT, -1, np.int64)
        perm[bin_of.astype(np.int64) * BIN + slot_of] = np.arange(NPC)
        valid = np.flatnonzero(perm >= 0)
        gids = k * NPC + perm[valid]
        ab = np.zeros((128, NSLOT), BF)
        ab[0:D, valid] = node_attr[gids].T.astype(BF)
        ab[D + ng_index[gids], valid] = 1.0

        m = {"edges_tok": edges_tok, "ridx": ridx, "ab": ab}
        m.update(shared)
        in_maps.append(m)
        perms.append(perm)
    return in_maps, perms


def _gather(outs, perms):
    full = np.zeros((N_NODES, OUT_DIM), np.float32)
    for k in range(NCORES):
        perm = perms[k]
        valid = np.flatnonzero(perm >= 0)
        full[k * NPC + perm[valid]] = np.asarray(outs[k])[:, valid].T
    return full


def kernel(**inputs):
    global _PROG
    if _PROG is None:
        _PROG = _build_program()
    in_maps, perms = _prep_inputs(**inputs)
    res = run_bass_kernel_spmd(_PROG, in_maps, list(range(NCORES)), trace=False)
    return _gather([res.results[k]["out"] for k in range(NCORES)], perms)


# revision 6
# speedup vs baseline: 1.4933x; 1.4933x over previous
"""GNN NodeBlock kernel for 8x TRN2 NeuronCores (v3).

v2 + smaller routing bins: nodes are packed into bins of 32 slots with
a fixed 3x128-token budget (LPT on degree, 27% slack), so the on-device
one-hot build touches 32 columns per token instead of 64 (DVE cost
-40%) and each aggregation matmul streams 32 one-hot columns (PE cost
-40%), for ~20% more edge-stream padding. Engine layout:
  SP/Pool: edge-payload DMA    Pool: ab DMA (SWDGE)
  DVE: one-hot EQ, aggT copy, output bias-add
  Act: relu, output DMA        PE: all matmuls
"""

import heapq

import ml_dtypes
import numpy as np
from contextlib import ExitStack

import concourse.bass as bass
import concourse.tile as tile
from concourse import bacc, mybir
from concourse.bass import AP
from concourse.bass_utils import run_bass_kernel_spmd

N_NODES = 100000
N_EDGES = 1000000
D = 64
NB = 64
LATENT = 32
OUT_DIM = 64

NCORES = 8
NPC = N_NODES // NCORES      # 12500 nodes per core
BIN = 32                     # node slots per bin
K3 = 3                       # 128-token tiles per bin
BTOK = K3 * 128              # 384 edge-token capacity per bin
NBIN = 416                   # bins per core
NSLOT = NBIN * BIN           # 13312 node slots (>= NPC)
NT = NBIN * K3               # 1248 token tiles per core
CAPT = NT * 128              # 159744 token slots per core
SUPB = 32                    # bins per supertile
SUP = SUPB * BIN             # 1024 nodes per supertile
WT = SUPB * K3               # 96 token tiles per supertile
NSUP = NBIN // SUPB          # 13 supertiles

F32 = mybir.dt.float32
F32R = mybir.dt.float32r
BF16 = mybir.dt.bfloat16
F8 = mybir.dt.float8e4
EQ = mybir.AluOpType.is_equal
ADD = mybir.AluOpType.add
Copy = mybir.ActivationFunctionType.Copy
Relu = mybir.ActivationFunctionType.Relu
Identity = mybir.ActivationFunctionType.Identity

_PROG = None

MACRO = 2        # supertiles per DMA macro-batch
# NOTE: GPSIMD/Pool cannot read PSUM on real HW and cannot run is_equal
# (the simulator allows both, neuronxcc rejects them) — Pool is DMA-only.
AGG_DVE = set()        # supertile aggT copies on DVE, rest on Act
OBT_ACT = set()        # supertile obT bias-adds on Act, rest on DVE
OBT_Q1_ACT = set()     # supertiles whose q=1 obT goes to Act
OUT_SP = set()         # macro indices whose output store issues from SP
OUT_POOL = set(range(8))  # macro indices whose output store issues from Pool
# (m0, nsup) DMA macro-batches; singles at both ends shorten startup/tail
MACROS = [(0, 1), (1, 2), (3, 2), (5, 2), (7, 2), (9, 2), (11, 1), (12, 1)]


def _bcast(ap, dim, n):
    """Insert a zero-stride dim of size n at free-dim position dim."""
    layout = list(ap.ap)
    layout.insert(1 + dim, [0, n])
    return AP(ap.tensor, ap.offset, layout)


def _build_program(reps=1):
    nc = bacc.Bacc(None, target_bir_lowering=False, debug=True)

    edges_d = nc.dram_tensor("edges_tok", [128, NT, D], F8, kind="ExternalInput")
    ridx_d = nc.dram_tensor("ridx", [128, NT], BF16, kind="ExternalInput")
    ab_d = nc.dram_tensor("ab", [128, NSLOT], BF16, kind="ExternalInput")
    gaT_d = nc.dram_tensor("gaT", [D, NB], F32R, kind="ExternalInput")
    w1n_d = nc.dram_tensor("w1n", [D, LATENT], BF16, kind="ExternalInput")
    w1a_d = nc.dram_tensor("w1a", [D, LATENT], BF16, kind="ExternalInput")
    w1g_d = nc.dram_tensor("w1g", [D, LATENT], F32R, kind="ExternalInput")
    w2_d = nc.dram_tensor("w2", [LATENT, OUT_DIM], F32R, kind="ExternalInput")
    b1_d = nc.dram_tensor("b1c", [LATENT, 1], F32, kind="ExternalInput")
    b2_d = nc.dram_tensor("b2b", [OUT_DIM, 512], F32, kind="ExternalInput")
    b2c_d = nc.dram_tensor("b2c", [OUT_DIM, 1], F32, kind="ExternalInput")
    iotax_d = nc.dram_tensor("iotax", [128, BIN, WT], BF16, kind="ExternalInput")
    out_d = nc.dram_tensor("out", [OUT_DIM, NSLOT], BF16, kind="ExternalOutput")

    with tile.TileContext(nc) as tc:
     # body repeated `reps` times for delta-timing (overhead cancels)
     for _rep in range(reps):
      with ExitStack() as stk:
        persist = stk.enter_context(tc.tile_pool(name="persist", bufs=1))
        gaT = persist.tile([D, NB], F32R)
        w1ng = persist.tile([128, LATENT], BF16)
        w1a = persist.tile([D, LATENT], BF16)
        w1g = persist.tile([D, LATENT], F32R)
        w2 = persist.tile([LATENT, OUT_DIM], F32R)
        b1c = persist.tile([LATENT, 1], F32)
        b2b = persist.tile([OUT_DIM, 512], F32)
        b2c = persist.tile([OUT_DIM, 1], F32)
        iotax = persist.tile([128, BIN, WT], BF16)
        ridx_sb = persist.tile([128, NT], BF16)

        # one-time const loads; ridx/iotax first (they gate the EQ chain).
        # ridx is split so supertile 0's slice lands immediately.
        nc.sync.dma_start(ridx_sb[:, 0:WT], ridx_d[:, 0:WT])
        nc.scalar.dma_start(ridx_sb[:, WT:], ridx_d[:, WT:])
        for eng, sb, dr in ((nc.gpsimd, iotax, iotax_d),
                            (nc.sync, gaT, gaT_d), (nc.sync, w1g, w1g_d),
                            (nc.scalar, w1a, w1a_d), (nc.scalar, w2, w2_d),
                            (nc.scalar, b1c, b1_d),
                            (nc.gpsimd, b2b, b2_d),
                            (nc.gpsimd, b2c, b2c_d)):
            eng.dma_start(sb[:], dr[:])
        nc.gpsimd.dma_start(w1ng[0:D, :], w1n_d[:])

        # G = global_attr @ W1g stacked under W1n as the [128,32] stationary
        with tc.tile_pool(name="psg", bufs=1, space="PSUM") as psg:
            ps_g = psg.tile([NB, LATENT], F32)
            nc.tensor.matmul(ps_g[:], gaT[:], w1g[:], start=True, stop=True)
            nc.scalar.activation(w1ng[D:128, :], ps_g[:], Copy)

        ppool = stk.enter_context(tc.tile_pool(name="pt", bufs=2))
        ohpool = stk.enter_context(tc.tile_pool(name="ohb", bufs=6))
        abpool = stk.enter_context(tc.tile_pool(name="ab", bufs=2))
        aggp = stk.enter_context(tc.tile_pool(name="agg", bufs=2))
        hp = stk.enter_context(tc.tile_pool(name="hp", bufs=2))
        opool = stk.enter_context(tc.tile_pool(name="op", bufs=2))
        psa = stk.enter_context(tc.tile_pool(name="psa", bufs=2, space="PSUM"))
        ps1p = stk.enter_context(tc.tile_pool(name="ps1", bufs=2, space="PSUM"))
        ps2p = stk.enter_context(tc.tile_pool(name="ps2", bufs=2, space="PSUM"))

        # macro-iterations: batches per-DMA fixed cost across supertiles.
        # First macro is a single supertile so the compute chain starts
        # as early as possible.
        macros = MACROS
        for mi, (m0, nsup) in enumerate(macros):
            mwt = nsup * WT
            msup = nsup * SUP
            ab = abpool.tile([128, msup], BF16, name="ab")
            nc.gpsimd.dma_start(ab[:], ab_d[:, SUP * m0:SUP * m0 + msup])

            pt = ppool.tile([128, mwt, D], F8, name="pt")
            nc.sync.dma_start(pt[:], edges_d[:, WT * m0:WT * m0 + mwt, :])

            obT = opool.tile([OUT_DIM, msup], BF16, name="obT")
            # one-hot routing, slot-major [128, BIN, WT]: every operand has
            # a stride-1 2-byte last dim, so DVE runs at 2 elem/cyc. All of
            # the macro's EQs are emitted first so DVE's in-order queue
            # doesn't stall them behind obT adds waiting on ps2.
            ohbTs = []
            for si in range(nsup):
                s = m0 + si
                ohbT = ohpool.tile([128, BIN, WT], BF16, name="ohbT")
                nc.vector.tensor_tensor(
                    ohbT[:], iotax[:],
                    _bcast(ridx_sb[:, WT * s:WT * (s + 1)], 0, BIN), op=EQ)
                ohbTs.append(ohbT)
            for si in range(nsup):
                s = m0 + si
                ohbT = ohbTs[si]

                # aggT[feat, node] accumulated per bin into one PSUM tile
                # (lhsT fp8 payload x strided bf16 one-hot columns)
                ps_agg = psa.tile([D, SUP], F32, name="ps_agg")
                for bi in range(SUPB):
                    for j in range(K3):
                        t = K3 * bi + j
                        nc.tensor.matmul(
                            ps_agg[:, BIN * bi:BIN * (bi + 1)],
                            pt[:, WT * si + t, :], ohbT[:, :, t],
                            start=(j == 0), stop=(j == K3 - 1))
                aggT = aggp.tile([D, SUP], BF16, name="aggT")
                if s in AGG_DVE:
                    nc.vector.tensor_scalar_add(aggT[:], ps_agg[:], 0.0)
                else:
                    nc.scalar.activation(aggT[:], ps_agg[:], Copy)

                for q in range(2):
                    sl = slice(SUP * si + 512 * q, SUP * si + 512 * (q + 1))
                    hsl = slice(512 * q, 512 * (q + 1))
                    ps1 = ps1p.tile([LATENT, 512], F32, name="ps1")
                    nc.tensor.matmul(ps1[:], w1ng[:], ab[:, sl],
                                     start=True, stop=False)
                    nc.tensor.matmul(ps1[:], w1a[:], aggT[:, hsl],
                                     start=False, stop=True)
                    h = hp.tile([LATENT, 512], F32R, name="h")
                    nc.scalar.activation(h[:], ps1[:], Relu, bias=b1c[:])
                    ps2 = ps2p.tile([OUT_DIM, 512], F32, name="ps2")
                    nc.tensor.matmul(ps2[:], w2[:], h[:], start=True, stop=True)
                    if s in OBT_ACT or (s in OBT_Q1_ACT and q == 1):
                        nc.scalar.activation(obT[:, sl], ps2[:], Identity,
                                             bias=b2c[:])
                    else:
                        nc.vector.tensor_tensor(obT[:, sl], ps2[:], b2b[:],
                                                op=ADD)
            out_eng = (nc.sync if mi in OUT_SP
                       else nc.gpsimd if mi in OUT_POOL else nc.scalar)
            out_eng.dma_start(out_d[:, SUP * m0:SUP * m0 + msup], obT[:])

    nc.compile()
    return nc


def _pack_bins(deg):
    """LPT bin-packing: assign each node to a bin, balancing edge load
    with caps of BIN nodes / BTOK edges per bin."""
    bin_of = np.empty(NPC, np.int32)
    slot_of = np.empty(NPC, np.int32)
    counts = np.zeros(NBIN, np.int32)
    loads = np.zeros(NBIN, np.int64)
    heap = [(0, b) for b in range(NBIN)]
    for n in np.argsort(-deg, kind="stable"):
        while True:
            load, b = heapq.heappop(heap)
            if counts[b] < BIN and loads[b] + deg[n] <= BTOK:
                break
        bin_of[n] = b
        slot_of[n] = counts[b]
        counts[b] += 1
        loads[b] += deg[n]
        if counts[b] < BIN:
            heapq.heappush(heap, (int(loads[b]), b))
    return bin_of, slot_of


def _prep_inputs(node_attr, edge_attr, global_attr, W1, b1, W2, b2,
                 receivers_idx, ng_index):
    node_attr = np.asarray(node_attr, np.float32)
    edge_attr = np.asarray(edge_attr, np.float32)
    global_attr = np.asarray(global_attr, np.float32)
    W1 = np.asarray(W1, np.float32)
    b1 = np.asarray(b1, np.float32)
    W2 = np.asarray(W2, np.float32)
    b2 = np.asarray(b2, np.float32)
    receivers_idx = np.asarray(receivers_idx, np.int64)
    ng_index = np.asarray(ng_index, np.int64)

    BF = ml_dtypes.bfloat16
    F8np = mybir.dt.np(F8)
    shared = {
        "gaT": np.ascontiguousarray(global_attr.T),
        "w1n": np.ascontiguousarray(W1[0:D]).astype(BF),
        "w1a": np.ascontiguousarray(W1[D:2 * D]).astype(BF),
        "w1g": np.ascontiguousarray(W1[2 * D:3 * D]),
        "w2": np.ascontiguousarray(W2),
        "b1c": np.ascontiguousarray(b1.reshape(LATENT, 1)),
        "b2b": np.ascontiguousarray(
            np.broadcast_to(b2[:, None], (OUT_DIM, 512))).astype(np.float32),
        "iotax": np.ascontiguousarray(np.broadcast_to(
            np.arange(BIN, dtype=BF)[None, :, None], (128, BIN, WT))),
        "b2c": np.ascontiguousarray(b2.reshape(OUT_DIM, 1)),
    }

    deg_all = np.bincount(receivers_idx, minlength=N_NODES).astype(np.float32)
    escale = (1.0 / np.maximum(deg_all, 1.0))[receivers_idx]

    order = np.argsort(receivers_idx, kind="stable")
    sorted_recv = receivers_idx[order]
    bounds = np.searchsorted(sorted_recv, np.arange(0, N_NODES + 1, NPC))

    in_maps = []
    perms = []
    for k in range(NCORES):
        sel = order[bounds[k]:bounds[k + 1]]
        lrecv = (sorted_recv[bounds[k]:bounds[k + 1]] - k * NPC).astype(np.int64)
        e = sel.size
        deg = np.bincount(lrecv, minlength=NPC)
        bin_of, slot_of = _pack_bins(deg)

        ew = bin_of[lrecv].astype(np.int64)
        ord2 = np.argsort(ew, kind="stable")
        sel2 = sel[ord2]
        lrecv2 = lrecv[ord2]
        ew2 = ew[ord2]
        starts = np.searchsorted(ew2, np.arange(NBIN))
        pos = np.arange(e) - starts[ew2]
        assert e == 0 or pos.max() < BTOK
        tokslot = ew2 * BTOK + pos

        tok = np.zeros((CAPT, D), F8np)
        tok[tokslot] = (edge_attr[sel2] * escale[sel2][:, None]).astype(F8np)
        edges_tok = np.ascontiguousarray(
            tok.reshape(NT, 128, D).transpose(1, 0, 2))
        rx = np.full(CAPT, -1.0, BF)
        rx[tokslot] = slot_of[lrecv2].astype(BF)
        ridx = np.ascontiguousarray(rx.reshape(NT, 128).T)

        perm = np.full(NSLOT, -1, np.int64)
        perm[bin_of.astype(np.int64) * BIN + slot_of] = np.arange(NPC)
        valid = np.flatnonzero(perm >= 0)
        gids = k * NPC + perm[valid]
        ab = np.zeros((128, NSLOT), BF)
        ab[0:D, valid] = node_attr[gids].T.astype(BF)
        ab[D + ng_index[gids], valid] = 1.0

        m = {"edges_tok": edges_tok, "ridx": ridx, "ab": ab}
        m.update(shared)
        in_maps.append(m)
        perms.append(perm)
    return in_maps, perms


def _gather(outs, perms):
    full = np.zeros((N_NODES, OUT_DIM), np.float32)
    for k in range(NCORES):
        perm = perms[k]
        valid = np.flatnonzero(perm >= 0)
        full[k * NPC + perm[valid]] = np.asarray(outs[k])[:, valid].T
    return full


def kernel(**inputs):
    global _PROG
    if _PROG is None:
        _PROG = _build_program()
    in_maps, perms = _prep_inputs(**inputs)
    res = run_bass_kernel_spmd(_PROG, in_maps, list(range(NCORES)), trace=False)
    return _gather([res.results[k]["out"] for k in range(NCORES)], perms)
